# revision 19
# baseline (speedup 1.0000x reference)
"""Trainium2 Bass kernel for PairwiseInteractionModule.

Reference computation (per batch b of 128, M=64 tokens, D=64):
  e = pos_table[positions] + aa_table[amino_acids]            # [M, D]
  pair(i,j) = concat(e_i, e_j)                                # [M, M, 2D]
  h1 = gelu(pair @ W1 + b1); h2 = gelu(h1 @ W2 + b2)
  s  = h2 @ W3 + b3                                           # [M, M]
  out = triu(s, 1) + triu(s, 1).T

Kernel strategy (8 cores, data-parallel over batch, 16 batches/core):
  - concat-MLP split: pair @ W1 = e_i @ W1a + e_j @ W1b, so precompute
    U = E @ W1a + b1 and V = E @ W1b per batch, then
    h1(i,j) = gelu(U_i + V_j) built with broadcast APs on the vector engine.
  - Only the upper triangle is needed: enumerate pairs in 8 row-strips
    (strip s: i in [8s,8s+8), j in [8s,64)) -> 2304 of 4096 columns.
  - Two batches packed per 128 partitions (d on 64, batch pair on 2x64).
  - Layer 2: blockdiag(W2, W2) [128,128] matmul into PSUM, gelu on ACT.
  - Layer 3: per-strip matmul with a [128,16] selector lhsT accumulating
    into one [16,512] PSUM bank; rows (2s, 2s+1) hold strip s's scores.
  - Epilogue: reshape-DMA to [64,64] score tiles, (s+b3)*mask, PE
    transpose, add, DMA out.
"""

import os
import numpy as np

import concourse.bass as bass
import concourse.mybir as mybir
import concourse.tile as tile
from concourse import bacc
from concourse.bass_utils import run_bass_kernel_spmd

F32 = mybir.dt.float32
I32 = mybir.dt.int32
BF16 = mybir.dt.bfloat16

B, M, P, A, D = 128, 64, 1024, 21, 64
NCORES = 8
BL = B // NCORES            # 16 batches per core
NPACK = BL // 2             # 8 packs of 2 batches / 8 token groups of 128
WIDTHS = [64 - 8 * s for s in range(8)]          # strip j-widths
SEGS = [8 * w for w in WIDTHS]                   # columns per strip
OFFS = np.concatenate([[0], np.cumsum(SEGS)]).tolist()
COLS = OFFS[-1]                                  # 2304
Z2CHUNK = 768                                    # psum chunk for layer 2

USE_BF16 = os.environ.get("BASSK_BF16", "0") == "1"
GCOLS = int(os.environ.get("BASSK_GCOLS", "1"))  # idx cols per indirect DMA
DT_PAIR = BF16 if USE_BF16 else F32

LAST_RESULTS = None
_PROGRAM_CACHE = {}


def _build_program():
    nc = bacc.Bacc("TRN2", target_bir_lowering=False, debug=False)

    wdt = DT_PAIR
    pos_idx_d = nc.dram_tensor("pos_idx", [128, NPACK], I32, kind="ExternalInput")
    aa_idx_d = nc.dram_tensor("aa_idx", [128, NPACK], I32, kind="ExternalInput")
    pos_table_d = nc.dram_tensor("pos_table", [P, D], F32, kind="ExternalInput")
    aa_table_d = nc.dram_tensor("aa_table", [A, D], F32, kind="ExternalInput")
    w1a2_d = nc.dram_tensor("w1a2", [128, 128], wdt, kind="ExternalInput")
    w1b2_d = nc.dram_tensor("w1b2", [128, 128], wdt, kind="ExternalInput")
    w2pack_d = nc.dram_tensor("w2pack", [128, 128], wdt, kind="ExternalInput")
    w3s_d = nc.dram_tensor("w3s", [128, 128], wdt, kind="ExternalInput")
    b1pack_d = nc.dram_tensor("b1pack", [128, 1], F32, kind="ExternalInput")
    b2pack_d = nc.dram_tensor("b2pack", [128, 1], F32, kind="ExternalInput")
    b3col_d = nc.dram_tensor("b3col", [64, 1], F32, kind="ExternalInput")
    id128_d = nc.dram_tensor("id128", [128, 128], wdt, kind="ExternalInput")
    id64_d = nc.dram_tensor("id64", [64, 64], F32, kind="ExternalInput")
    mask64_d = nc.dram_tensor("mask64", [64, 64], F32, kind="ExternalInput")
    out_d = nc.dram_tensor("out", [BL, M, M], F32, kind="ExternalOutput")

    with tile.TileContext(nc) as tc:
        with (
            tc.tile_pool(name="const", bufs=1) as cpool,
            tc.tile_pool(name="emb", bufs=3) as epool,
            tc.tile_pool(name="uv", bufs=1) as uvpool,
            tc.tile_pool(name="pair", bufs=2) as ppool,
            tc.tile_pool(name="ep", bufs=2) as eppool,
            tc.tile_pool(name="ps_pro", bufs=2, space="PSUM") as ps_pro,
            tc.tile_pool(name="ps_z2", bufs=2, space="PSUM") as ps_z2,
            tc.tile_pool(name="ps_s", bufs=2, space="PSUM") as ps_s,
        ):
            # ---- constants into SBUF ----
            def cload(dram, shape, dtype, tag):
                t = cpool.tile(shape, dtype, tag=tag, name=tag)
                nc.sync.dma_start(t[:], dram[:])
                return t

            pos_idx = cload(pos_idx_d, [128, NPACK], I32, "pos_idx")
            aa_idx = cload(aa_idx_d, [128, NPACK], I32, "aa_idx")
            w1a2 = cload(w1a2_d, [128, 128], wdt, "w1a2")
            w1b2 = cload(w1b2_d, [128, 128], wdt, "w1b2")
            w2pack = cload(w2pack_d, [128, 128], wdt, "w2pack")
            w3s = cload(w3s_d, [128, 128], wdt, "w3s")
            b1pack = cload(b1pack_d, [128, 1], F32, "b1pack")
            b2pack = cload(b2pack_d, [128, 1], F32, "b2pack")
            b3col = cload(b3col_d, [64, 1], F32, "b3col")
            id128 = cload(id128_d, [128, 128], wdt, "id128")
            id64 = cload(id64_d, [64, 64], F32, "id64")
            mask64 = cload(mask64_d, [64, 64], F32, "mask64")

            u2sb = []
            v2sb = []

            # ---- prologue per token-group g (= pack g: batches 2g, 2g+1) ----
            for g in range(NPACK):
                pos_g = epool.tile([128, D], F32, tag="pos_g", name="pos_g")
                aa_g = epool.tile([128, D], F32, tag="aa_g", name="aa_g")
                nc.gpsimd.indirect_dma_start(
                    out=pos_g[:],
                    out_offset=None,
                    in_=pos_table_d[:],
                    in_offset=bass.IndirectOffsetOnAxis(
                        ap=pos_idx[:, g : g + 1], axis=0
                    ),
                )
                nc.gpsimd.indirect_dma_start(
                    out=aa_g[:],
                    out_offset=None,
                    in_=aa_table_d[:],
                    in_offset=bass.IndirectOffsetOnAxis(
                        ap=aa_idx[:, g : g + 1], axis=0
                    ),
                )
                e_g = epool.tile([128, D], wdt, tag="e_g", name="e_g")
                nc.vector.tensor_tensor(
                    out=e_g[:], in0=pos_g[:], in1=aa_g[:], op=mybir.AluOpType.add
                )
                # transpose [128 tok, 64 d] -> [64 d, 128 tok]
                etg_ps = ps_pro.tile([64, 128], wdt, tag="pro", name="etg_ps")
                nc.tensor.transpose(out=etg_ps[:], in_=e_g[:], identity=id128[:])
                etg = epool.tile([64, 128], wdt, tag="etg", name="etg")
                nc.vector.tensor_copy(out=etg[:], in_=etg_ps[:])
                # partition-stack the two 64-token halves -> [128, 64]
                et2 = epool.tile([128, 64], wdt, tag="et2", name="et2")
                nc.sync.dma_start(et2[0:64, :], etg[:, 0:64])
                nc.sync.dma_start(et2[64:128, :], etg[:, 64:128])
                # U2 = blockdiag(W1a).T @ et2 (+b1), V2 likewise
                u2_ps = ps_pro.tile([128, 64], F32, tag="pro", name="u2_ps")
                nc.tensor.matmul(
                    out=u2_ps[:], lhsT=w1a2[:], rhs=et2[:], start=True, stop=True
                )
                u2 = uvpool.tile([128, 64], wdt, tag=f"u2_{g}", name=f"u2_{g}")
                nc.vector.tensor_scalar_add(out=u2[:], in0=u2_ps[:], scalar1=b1pack[:])
                v2_ps = ps_pro.tile([128, 64], F32, tag="pro", name="v2_ps")
                nc.tensor.matmul(
                    out=v2_ps[:], lhsT=w1b2[:], rhs=et2[:], start=True, stop=True
                )
                v2 = uvpool.tile([128, 64], wdt, tag=f"v2_{g}", name=f"v2_{g}")
                nc.vector.tensor_copy(out=v2[:], in_=v2_ps[:])
                u2sb.append(u2)
                v2sb.append(v2)

            # ---- main loop per pack ----
            for k in range(NPACK):
                u2, v2 = u2sb[k], v2sb[k]
                h1_pre = ppool.tile([128, COLS], wdt, tag="h1_pre", name="h1_pre")
                # strip columns are (jl-major, il-minor) so the epilogue
                # scatter-DMA gets a legal partition-pitch step on il
                for s in range(8):
                    w = WIDTHS[s]
                    o = OFFS[s]
                    in0 = u2[:, 8 * s : 8 * s + 8].unsqueeze(1).broadcast_to(
                        [128, w, 8]
                    )
                    in1 = v2[:, 8 * s : 64].unsqueeze(2).broadcast_to([128, w, 8])
                    outap = h1_pre[:, o : o + 8 * w].rearrange(
                        "p (a b) -> p a b", a=w
                    )
                    nc.vector.tensor_tensor(
                        out=outap, in0=in0, in1=in1, op=mybir.AluOpType.add
                    )
                h1 = ppool.tile([128, COLS], wdt, tag="h1", name="h1")
                nc.scalar.activation(
                    h1[:], h1_pre[:], mybir.ActivationFunctionType.Gelu
                )
                h2 = ppool.tile([128, COLS], wdt, tag="h2", name="h2")
                for c0 in range(0, COLS, Z2CHUNK):
                    cw = min(Z2CHUNK, COLS - c0)
                    z2 = ps_z2.tile([128, Z2CHUNK], F32, tag="z2", name="z2")
                    for m0 in range(0, cw, 512):
                        mw = min(512, cw - m0)
                        nc.tensor.matmul(
                            out=z2[:, m0 : m0 + mw],
                            lhsT=w2pack[:],
                            rhs=h1[:, c0 + m0 : c0 + m0 + mw],
                            start=True,
                            stop=True,
                        )
                    nc.scalar.activation(
                        h2[:, c0 : c0 + cw],
                        z2[:, 0:cw],
                        mybir.ActivationFunctionType.Gelu,
                        bias=b2pack[:],
                    )
                # layer 3: accumulate strip scores into one [16, 512] bank
                s_all = ps_s.tile([16, 512], F32, tag="s_all", name="s_all")
                for s in range(8):
                    seg = SEGS[s]
                    nc.tensor.matmul(
                        out=s_all[:, 0:seg],
                        lhsT=w3s[:, 16 * s : 16 * s + 16],
                        rhs=h2[:, OFFS[s] : OFFS[s] + seg],
                        start=(s == 0),
                        stop=(s == 7),
                    )
                # DMA cannot read PSUM: bounce scores to SBUF first
                s_sb = eppool.tile([16, 512], F32, tag="s_sb", name="s_sb")
                nc.vector.tensor_copy(out=s_sb[:], in_=s_all[:])
                # ---- epilogue per batch in the pack ----
                # stT[j, i] = s[i, j]: strip s's (jl-major) columns land as a
                # contiguous [1, 8w] src -> [w part, 8 col] dst (legal APs).
                for bloc in range(2):
                    st = eppool.tile([64, 64], F32, tag="st", name="st")
                    nc.vector.memset(st[:], 0.0)
                    for s in range(8):
                        w = WIDTHS[s]
                        src = s_sb[2 * s + bloc : 2 * s + bloc + 1, 0 : 8 * w]
                        dst = st[8 * s : 64, 8 * s : 8 * s + 8]
                        nc.sync.dma_start(dst, src)
                    # masked^T = (s^T + b3) * tril-mask
                    masked = eppool.tile([64, 64], F32, tag="masked", name="masked")
                    nc.vector.scalar_tensor_tensor(
                        out=masked[:],
                        in0=st[:],
                        scalar=b3col[:],
                        in1=mask64[:],
                        op0=mybir.AluOpType.add,
                        op1=mybir.AluOpType.mult,
                    )
                    # out = transpose(masked^T) + masked^T
                    mt_ps = ps_pro.tile([64, 64], F32, tag="pro", name="mt_ps")
                    nc.tensor.transpose(
                        out=mt_ps[:], in_=masked[:], identity=id64[:]
                    )
                    ob = eppool.tile([64, 64], F32, tag="ob", name="ob")
                    nc.vector.tensor_tensor(
                        out=ob[:], in0=masked[:], in1=mt_ps[:],
                        op=mybir.AluOpType.add,
                    )
                    nc.sync.dma_start(out_d[2 * k + bloc, :, :], ob[:])

    nc.compile()
    return nc


def _get_program():
    key = (USE_BF16, GCOLS)
    if key not in _PROGRAM_CACHE:
        _PROGRAM_CACHE[key] = _build_program()
    return _PROGRAM_CACHE[key]


def _host_prep(positions, amino_acids, pos_table, aa_table, W1, b1, W2, b2, W3, b3):
    import ml_dtypes

    f32 = np.float32
    pos = np.clip(np.asarray(positions), 0, P - 1).astype(np.int32)
    aa = np.clip(np.asarray(amino_acids), 0, A - 1).astype(np.int32)
    pos_table = np.ascontiguousarray(np.asarray(pos_table, dtype=f32))
    aa_table = np.ascontiguousarray(np.asarray(aa_table, dtype=f32))
    W1 = np.asarray(W1, dtype=f32)
    W2 = np.asarray(W2, dtype=f32)
    W3 = np.asarray(W3, dtype=f32)
    b1 = np.asarray(b1, dtype=f32)
    b2 = np.asarray(b2, dtype=f32)
    b3 = np.asarray(b3, dtype=f32)

    wdt = ml_dtypes.bfloat16 if USE_BF16 else f32

    def blockdiag(a):
        z = np.zeros((128, 128), dtype=f32)
        z[0:64, 0:64] = a
        z[64:128, 64:128] = a
        return np.ascontiguousarray(z.astype(wdt))

    w1a2 = blockdiag(W1[:64])
    w1b2 = blockdiag(W1[64:])
    w2pack = blockdiag(W2)
    w3s = np.zeros((128, 8, 16), dtype=f32)
    for s in range(8):
        w3s[0:64, s, 2 * s] = W3[:, 0]
        w3s[64:128, s, 2 * s + 1] = W3[:, 0]
    w3s = np.ascontiguousarray(w3s.reshape(128, 128).astype(wdt))

    b1pack = np.concatenate([b1, b1]).reshape(128, 1).astype(f32)
    b2pack = np.concatenate([b2, b2]).reshape(128, 1).astype(f32)
    b3col = np.full((64, 1), float(b3.reshape(-1)[0]), dtype=f32)
    id128 = np.ascontiguousarray(np.eye(128, dtype=f32).astype(wdt))
    id64 = np.eye(64, dtype=f32)
    # tril mask: stT holds transposed scores, keep entries with row(j) > col(i)
    mask64 = (np.arange(64)[:, None] > np.arange(64)[None, :]).astype(f32)

    shared = dict(
        pos_table=pos_table, aa_table=aa_table,
        w1a2=w1a2, w1b2=w1b2, w2pack=w2pack, w3s=w3s,
        b1pack=b1pack, b2pack=b2pack, b3col=b3col,
        id128=id128, id64=id64, mask64=mask64,
    )

    in_maps = []
    for c in range(NCORES):
        ptok = pos[BL * c : BL * (c + 1)].reshape(-1)      # [1024]
        atok = aa[BL * c : BL * (c + 1)].reshape(-1)
        m = dict(shared)
        m["pos_idx"] = np.ascontiguousarray(ptok.reshape(NPACK, 128).T)
        m["aa_idx"] = np.ascontiguousarray(atok.reshape(NPACK, 128).T)
        in_maps.append(m)
    return in_maps


class _Runner:
    """Persistent PJRT executor for the Bass program (axon path), with a
    chained-N variant for measuring per-execution device time."""

    def __init__(self, nc):
        import jax
        from jax.sharding import Mesh, PartitionSpec
        from jax.experimental.shard_map import shard_map
        import concourse.mybir as mybir_
        from concourse import bass2jax

        bass2jax.install_neuronx_cc_hook()
        self._bass_exec_p = bass2jax._bass_exec_p
        self.nc = nc
        partition_name = (
            nc.partition_id_tensor.name if nc.partition_id_tensor else None
        )
        partition_id_tensor = bass2jax.partition_id_tensor
        in_names, out_names, out_avals, zero_outs = [], [], [], []
        for alloc in nc.m.functions[0].allocations:
            if not isinstance(alloc, mybir_.MemoryLocationSet):
                continue
            name = alloc.memorylocations[0].name
            if alloc.kind == "ExternalInput":
                if name != partition_name:
                    in_names.append(name)
            elif alloc.kind == "ExternalOutput":
                shape = tuple(alloc.tensor_shape)
                dtype = mybir_.dt.np(alloc.dtype)
                out_names.append(name)
                out_avals.append(jax.core.ShapedArray(shape, dtype))
                zero_outs.append(np.zeros(shape, dtype))
        self.in_names = in_names
        self.out_names = out_names
        self.zero_outs = zero_outs
        self.n_params = len(in_names)
        n_outs = len(out_names)
        bind_in_names = tuple(
            in_names + out_names + ([partition_name] if partition_name else [])
        )
        out_avals_t = tuple(out_avals)

        devices = jax.devices()[:NCORES]
        mesh = Mesh(np.asarray(devices), ("core",))
        donate = tuple(range(self.n_params, self.n_params + n_outs))
        exec_p = self._bass_exec_p

        def body_n(niter):
            def _body(*args):
                ins = list(args[: self.n_params])
                outs = list(args[self.n_params :])
                pid = [partition_id_tensor()] if partition_name else []
                for _ in range(niter):
                    outs = list(
                        exec_p.bind(
                            *ins,
                            *outs,
                            *pid,
                            out_avals=out_avals_t,
                            in_names=bind_in_names,
                            out_names=tuple(out_names),
                            lowering_input_output_aliases=(),
                            sim_require_finite=True,
                            sim_require_nnan=True,
                            nc=nc,
                        )
                    )
                return tuple(outs)

            return jax.jit(
                shard_map(
                    _body,
                    mesh=mesh,
                    in_specs=(PartitionSpec("core"),) * (self.n_params + n_outs),
                    out_specs=(PartitionSpec("core"),) * n_outs,
                    check_rep=False,
                ),
                donate_argnums=donate,
                keep_unused=True,
            )

        self._jit_cache = {}
        self._body_n = body_n

    def run(self, in_maps, niter=1):
        # the neuronx hook supports one bass_exec per module; chain at the
        # python level instead (run_chain)
        assert niter == 1
        fn = self._jit_cache.setdefault(1, self._body_n(1))
        concat_in = [
            np.concatenate([np.asarray(m[name]) for m in in_maps], axis=0)
            for name in self.in_names
        ]
        concat_zero = [
            np.concatenate([z] * len(in_maps), axis=0) for z in self.zero_outs
        ]
        outs = fn(*concat_in, *concat_zero)
        outs = [np.asarray(o) for o in outs]
        per_core = []
        for c in range(len(in_maps)):
            d = {}
            for i, name in enumerate(self.out_names):
                n0 = self.zero_outs[i].shape[0]
                d[name] = outs[i][c * n0 : (c + 1) * n0]
            per_core.append(d)
        return per_core


_RUNNER = None


def _get_runner():
    global _RUNNER
    if _RUNNER is None:
        _RUNNER = _Runner(_get_program())
    return _RUNNER


def kernel(**inputs) -> np.ndarray:
    in_maps = _host_prep(**inputs)
    results = _get_runner().run(in_maps, niter=1)
    out = np.concatenate([r["out"] for r in results], axis=0)
    return np.ascontiguousarray(out.astype(np.float32))


def bench(inputs, n1=8, n2=56, reps=3):
    """Estimate per-execution device time: launch N chained executions
    (outputs feed the next call's donated out-buffers, all async) and take
    the slope between two chain lengths."""
    import time
    import jax

    r = _get_runner()
    in_maps = _host_prep(**inputs)
    fn = r._jit_cache.setdefault(1, r._body_n(1))
    concat_in = [
        np.concatenate([np.asarray(m[name]) for m in in_maps], axis=0)
        for name in r.in_names
    ]
    concat_zero = [np.concatenate([z] * len(in_maps), axis=0) for z in r.zero_outs]
    ins_dev = [jax.device_put(x) for x in concat_in]

    def chain(n):
        outs = [jax.device_put(z) for z in concat_zero]
        for o in outs:
            o.block_until_ready()
        t0 = time.perf_counter()
        for _ in range(n):
            outs = list(fn(*ins_dev, *outs))
        for o in outs:
            o.block_until_ready()
        return time.perf_counter() - t0

    chain(2)  # warm
    t1 = min(chain(n1) for _ in range(reps))
    t2 = min(chain(n2) for _ in range(reps))
    per_exec = (t2 - t1) / (n2 - n1)
    return per_exec, t1, t2
